# revision 19
# baseline (speedup 1.0000x reference)
"""ConvGRU Trainium2 kernel (8 NeuronCores, data-parallel over batch x H-half).

v3: pair-granular schedule (2 pipeline stages of 784 px), multi-bank PSUM
tiles with 3D-AP activations, col-paired M=64 c-gate matmuls, partition-offset
engine ops instead of realign DMAs, batched DMA, and explicitly staged
per-engine emission order (engines are in-order; program order must match
operand readiness to avoid head-of-line stalls).

Layout (per core shard: batch b, H rows [hh*28, hh*28+28), N = 28*56 = 1568 px):
  Channels on SBUF partitions, pixels on the free dim. C=192 = 128 "low"
  (p0:128 of full tiles) + 64 "high" (p64:128 of mixed tiles).
  Mixed tiles: M1 = [x128:192 @p0:64 ; h128:192 @p64:128],
               M2 = [x128:192 @p0:64 ; rh128:192 @p64:128].
  zr matmul M-tiles: m0=z0:128, m1=r0:128, m2=[z128:192 @p0:64; r128:192 @p64:128].
  c-gate: psa (M=128, c0:128); cb of the two lanes of a pair col-paired into
  ONE bank: lane A -> p64:128 (tile_position (0,64)), lane B -> p0:64 ((0,0)).
  z128:192 (at p0:64 of ZRB) is re-aligned to p64:128 once per pair by a Pool
  tensor_copy with output partition offset.

PSUM budget (8 banks): shared zr+ca pool 3x[128,2,512] (6 banks) + cb pool
2x[128,512] (2 banks).

All on-device tensors fp16 (PSUM fp32); rel err vs fp32 reference ~1.6e-3.
"""

import sys

sys.path.insert(0, "/opt/trn_rl_repo")

import numpy as np

B, T, C, H, W = 4, 16, 192, 56, 56
HH = 28          # H rows per shard
N = HH * W       # 1568 pixels per shard per step
LANE = 392       # pixels per matmul N-chunk (one PSUM bank holds 512 fp32)
PAIR = 2 * LANE  # 784: elementwise/ACT granularity
NCORES = 8

_CACHE = {}


def _build_nc(n_steps):
    from concourse import bacc
    import concourse.mybir as mybir
    import concourse.tile as tile

    F16, F32 = mybir.dt.float16, mybir.dt.float32
    AF = mybir.ActivationFunctionType

    nc = bacc.Bacc(None, target_bir_lowering=False)
    x_d = nc.dram_tensor("x", [n_steps, C, N], F16, kind="ExternalInput")
    wzr_d = nc.dram_tensor("wzr", [128, 3, 3, 128], F16, kind="ExternalInput")
    wc_d = nc.dram_tensor("wc", [128, 3, 192], F16, kind="ExternalInput")
    bzr_d = nc.dram_tensor("bzr", [128, 3], F32, kind="ExternalInput")
    bc_d = nc.dram_tensor("bc", [128, 2], F32, kind="ExternalInput")
    y_d = nc.dram_tensor("y", [n_steps, C, N], F16, kind="ExternalOutput")

    with tile.TileContext(nc) as tc:
        with (
            tc.tile_pool(name="const", bufs=1) as constp,
            tc.tile_pool(name="state", bufs=1) as statep,
            tc.tile_pool(name="pszr", bufs=3, space="PSUM") as pszr,
            tc.tile_pool(name="pscb", bufs=2, space="PSUM") as pscb,
        ):
            # load order: what the first matmuls/ACTs need first.
            # wzr split per M-tile (m1 first): the first Ldweights only
            # needs the m1 slice.
            wzr = constp.tile([128, 3, 3, 128], F16)
            for m in (1, 2, 0):
                nc.sync.dma_start(wzr[:, :, m, :], wzr_d[:, :, m, :])

            Xb = [
                statep.tile([128, N], F16, name="X0a"),
                statep.tile([128, N], F16, name="X0b"),
            ]
            H0 = statep.tile([128, N], F16, name="H0")    # h 0:128
            M1 = statep.tile([128, N], F16, name="M1")    # [x-up; h-up]
            M2 = statep.tile([128, N], F16, name="M2")    # [x-up; rh-up]
            Z0 = statep.tile([128, N], F16, name="Z0")    # z 0:128
            R0 = statep.tile([128, N], F16, name="R0")    # r 0:128
            # ZRB per (parity, pair): the Pool z-copy reads it late, and the
            # dep tracker is conservative across column ranges — separate
            # tiles kill both the WAR edge and the false cross-pair dep
            ZRBb = [
                [
                    statep.tile([128, PAIR], F16, name=f"ZRB{i}{p}")
                    for p in range(2)
                ]
                for i in range(2)
            ]
            ZU = statep.tile([128, N], F16, name="ZU")    # z-up aligned @p64:128
            RH0 = statep.tile([128, N], F16, name="RH0")  # r*h 0:128
            CA = statep.tile([128, N], F16, name="CA")    # c 0:128
            UC = statep.tile([128, N], F16, name="UC")    # c-up @p64:128
            D = statep.tile([128, N], F16, name="D")      # scratch (low)
            EU = statep.tile([128, N], F16, name="EU")    # scratch (up)

            nc.gpsimd.dma_start(Xb[0][:], x_d[0, 0:128, :])
            nc.gpsimd.dma_start(M1[0:64, :], x_d[0, 128:192, :])

            bzr = constp.tile([128, 3], F32)
            nc.sync.dma_start(bzr[:], bzr_d[:])
            wc = constp.tile([128, 3, 192], F16)
            nc.sync.dma_start(wc[:], wc_d[:])
            nc.sync.dma_start(M2[0:64, :], x_d[0, 128:192, :])
            bc = constp.tile([128, 2], F32)
            nc.sync.dma_start(bc[:], bc_d[:])

            nc.vector.memset(H0[:], 0.0)
            nc.vector.memset(M1[64:128, :], 0.0)
            nc.vector.memset(M2[64:128, :], 0.0)

            def zr_phase(t, p, X0, ZRB):
                p0 = p * PAIR
                psl = slice(p0, p0 + PAIR)
                lsl = [slice(p0, p0 + LANE), slice(p0 + LANE, p0 + PAIR)]
                # M-tile order (m1, m2, m0): r-activations drain first — they
                # feed rh which gates the c-phase; z (m0) is only needed at
                # the late h-update.
                for m in (1, 2, 0):
                    ps = pszr.tile([128, 2, 512], F32, tag="zrca")
                    # t=0: h is zero and ready (memset) while x still loads —
                    # lead with H0 zero-chunks so PE warms up during the DMA
                    korder = ((0, H0), (1, M1), (2, X0)) if t == 0 else (
                        (2, X0), (0, H0), (1, M1))
                    first, last = korder[0][1], korder[-1][1]
                    for k, src in korder:
                        for li in range(2):
                            nc.tensor.matmul(
                                ps[:, li, 0:LANE],
                                wzr[:, k, m, :],
                                src[:, lsl[li]],
                                start=(src is first),
                                stop=(src is last),
                                skip_group_check=True,
                            )
                    if m == 2:
                        nc.scalar.activation(
                            ZRB[:, :], ps[:, :, 0:LANE], AF.Sigmoid,
                            bias=bzr[:, 2:3], scale=1.0,
                        )
                    else:
                        dst = (Z0, R0)[m]
                        nc.scalar.activation(
                            dst[:, psl], ps[:, :, 0:LANE], AF.Sigmoid,
                            bias=bzr[:, m : m + 1], scale=1.0,
                        )
                if p == 1 and t + 1 < n_steps:
                    # all zr readers of M1[0:64] emitted; reload x-half for t+1
                    nc.sync.dma_start(M1[0:64, :], x_d[t + 1, 128:192, :])

            def rh_phase(t, p, ZRB):
                p0 = p * PAIR
                psl = slice(p0, p0 + PAIR)
                if t > 0:
                    # at t=0 h==0 so rh==0 (M2 upper half is pre-zeroed)
                    nc.vector.tensor_mul(
                        out=RH0[:, psl], in0=R0[:, psl], in1=H0[:, psl]
                    )
                    nc.vector.tensor_mul(
                        out=M2[64:128, psl], in0=ZRB[64:128, :],
                        in1=M1[64:128, psl],
                    )
                if t + 1 < n_steps:
                    # z-up realign p0:64 -> p64:128 (Pool, off critical path).
                    # Last step uses the PSUM-d variant instead (no Pool hop
                    # in the drain chain).
                    nc.gpsimd.tensor_copy(out=ZU[64:128, psl], in_=ZRB[0:64, :])

            def ca_act(p, psa):
                p0 = p * PAIR
                psl = slice(p0, p0 + PAIR)
                nc.scalar.activation(
                    CA[:, psl], psa[:, :, 0:LANE], AF.Tanh,
                    bias=bc[:, 0:1], scale=1.0,
                )

            def cb_acts(p, psb):
                p0 = p * PAIR
                lsl = [slice(p0, p0 + LANE), slice(p0 + LANE, p0 + PAIR)]
                nc.scalar.activation(
                    UC[64:128, lsl[0]], psb[64:128, 0:LANE], AF.Tanh,
                    bias=bc[64:128, 1:2], scale=1.0,
                )
                nc.scalar.activation(
                    UC[64:128, lsl[1]], psb[0:64, 0:LANE], AF.Tanh,
                    bias=bc[0:64, 1:2], scale=1.0,
                )

            def c_mms(t, p, X0):
                p0 = p * PAIR
                lsl = [slice(p0, p0 + LANE), slice(p0 + LANE, p0 + PAIR)]
                psa = pszr.tile([128, 2, 512], F32, tag="zrca")
                psb = pscb.tile([128, 512], F32, tag="cb")
                korder = ((2, X0), (1, M2)) if t == 0 else (
                    (2, X0), (0, RH0), (1, M2))
                for k, src in korder:
                    for li in range(2):
                        rhs = src[:, lsl[li]]
                        nc.tensor.matmul(
                            psa[:, li, 0:LANE], wc[:, k, 0:128], rhs,
                            start=(src is X0), stop=(src is M2),
                            skip_group_check=True,
                        )
                        if li == 0:
                            nc.tensor.matmul(
                                psb[64:128, 0:LANE], wc[:, k, 128:192], rhs,
                                start=(src is X0), stop=(src is M2),
                                skip_group_check=True,
                                tile_position=(0, 64),
                            )
                        else:
                            nc.tensor.matmul(
                                psb[0:64, 0:LANE], wc[:, k, 128:192], rhs,
                                start=(src is X0), stop=(src is M2),
                                skip_group_check=True,
                                tile_position=(0, 0),
                            )
                if p == 1 and t + 1 < n_steps:
                    # c readers of M2[0:64] emitted; reload x-half for t+1
                    nc.sync.dma_start(M2[0:64, :], x_d[t + 1, 128:192, :])
                return psa, psb

            def h_low(p):
                p0 = p * PAIR
                psl = slice(p0, p0 + PAIR)
                nc.vector.tensor_sub(
                    out=D[:, psl], in0=CA[:, psl], in1=H0[:, psl]
                )
                nc.vector.tensor_mul(
                    out=D[:, psl], in0=Z0[:, psl], in1=D[:, psl]
                )
                nc.vector.tensor_add(
                    out=H0[:, psl], in0=H0[:, psl], in1=D[:, psl]
                )

            def h_up_sub(p):
                p0 = p * PAIR
                psl = slice(p0, p0 + PAIR)
                nc.vector.tensor_sub(
                    out=EU[64:128, psl], in0=UC[64:128, psl],
                    in1=M1[64:128, psl],
                )

            def h_up_fin(p):
                p0 = p * PAIR
                psl = slice(p0, p0 + PAIR)
                nc.vector.tensor_mul(
                    out=EU[64:128, psl], in0=ZU[64:128, psl],
                    in1=EU[64:128, psl],
                )
                nc.vector.tensor_add(
                    out=M1[64:128, psl], in0=M1[64:128, psl],
                    in1=EU[64:128, psl],
                )

            def h_up_psumd(p, psb, ZRB):
                # tail variant: d lives in psb (PSUM) so the z-up multiply is
                # a legal mixed-space op — no Pool z-copy in the chain
                p0 = p * PAIR
                psl = slice(p0, p0 + PAIR)
                lsl = [slice(p0, p0 + LANE), slice(p0 + LANE, p0 + PAIR)]
                for li, pr in ((0, slice(64, 128)), (1, slice(0, 64))):
                    nc.vector.tensor_sub(
                        out=psb[pr, 0:LANE], in0=UC[64:128, lsl[li]],
                        in1=M1[64:128, lsl[li]],
                    )
                    nc.vector.tensor_mul(
                        out=EU[64:128, lsl[li]], in0=psb[pr, 0:LANE],
                        in1=ZRB[0:64, li * LANE:(li + 1) * LANE],
                    )
                nc.vector.tensor_add(
                    out=M1[64:128, psl], in0=M1[64:128, psl],
                    in1=EU[64:128, psl],
                )

            for t in range(n_steps):
                X0 = Xb[t % 2]
                if t + 1 < n_steps:
                    nc.sync.dma_start(Xb[(t + 1) % 2][:], x_d[t + 1, 0:128, :])

                zr_phase(t, 0, X0, ZRBb[t % 2][0])
                rh_phase(t, 0, ZRBb[t % 2][0])
                psa0, psb0 = c_mms(t, 0, X0)
                ca_act(0, psa0)
                zr_phase(t, 1, X0, ZRBb[t % 2][1])
                rh_phase(t, 1, ZRBb[t % 2][1])
                cb_acts(0, psb0)
                h_low(0)
                if t + 1 < n_steps:
                    h_up_sub(0)
                    h_up_fin(0)
                else:
                    h_up_psumd(0, psb0, ZRBb[t % 2][0])
                psa1, psb1 = c_mms(t, 1, X0)
                ca_act(1, psa1)
                cb_acts(1, psb1)
                h_low(1)
                if t + 1 < n_steps:
                    h_up_sub(1)
                    h_up_fin(1)
                else:
                    h_up_psumd(1, psb1, ZRBb[t % 2][1])

                if t + 1 < n_steps:
                    nc.sync.dma_start(y_d[t, 0:128, :], H0[:])
                    nc.sync.dma_start(y_d[t, 128:192, :], M1[64:128, :])
                else:
                    # final step: stream out per pair to shorten the tail
                    for p in range(2):
                        psl = slice(p * PAIR, (p + 1) * PAIR)
                        nc.sync.dma_start(y_d[t, 0:128, psl], H0[:, psl])
                        nc.sync.dma_start(y_d[t, 128:192, psl], M1[64:128, psl])

    nc.finalize()
    return nc


def _prep_weights(w_z, w_r, w_h, b_z, b_r, b_h):
    """Host-side weight/bias packing to match the device layout."""
    wz = np.asarray(w_z, np.float32)
    wr = np.asarray(w_r, np.float32)
    wh = np.asarray(w_h, np.float32)

    def k_blocks(Wm):
        # K-tile order (H-part, mixed, X-part) matching rhs tiles (H0, M1, X0)
        return [
            Wm[:, 192:320],
            np.concatenate([Wm[:, 128:192], Wm[:, 320:384]], axis=1),
            Wm[:, 0:128],
        ]

    m_blocks = [
        wz[0:128],
        wr[0:128],
        np.concatenate([wz[128:192], wr[128:192]], axis=0),
    ]
    wzr = np.zeros((128, 3, 3, 128), np.float16)
    for m, Wm in enumerate(m_blocks):
        for k, Wk in enumerate(k_blocks(Wm)):
            wzr[:, k, m, :] = Wk.T.astype(np.float16)

    wc = np.zeros((128, 3, 192), np.float16)
    for k, Wk in enumerate(k_blocks(wh)):
        wc[:, k, :] = Wk.T.astype(np.float16)

    bzr = np.zeros((128, 3), np.float32)
    bzr[:, 0] = b_z[0:128]
    bzr[:, 1] = b_r[0:128]
    bzr[0:64, 2] = b_z[128:192]
    bzr[64:128, 2] = b_r[128:192]
    bc = np.zeros((128, 2), np.float32)
    bc[:, 0] = b_h[0:128]
    bc[0:64, 1] = b_h[128:192]
    bc[64:128, 1] = b_h[128:192]
    return wzr, wc, bzr, bc


def _shards():
    return [(b, hh) for b in range(B) for hh in range(2)]


def kernel(**inputs):
    video = np.asarray(inputs["video"], np.float32)
    wzr, wc, bzr, bc = _prep_weights(
        inputs["w_z"], inputs["w_r"], inputs["w_h"],
        np.asarray(inputs["b_z"], np.float32),
        np.asarray(inputs["b_r"], np.float32),
        np.asarray(inputs["b_h"], np.float32),
    )

    if "nc" not in _CACHE:
        _CACHE["nc"] = _build_nc(T)
    nc = _CACHE["nc"]

    in_maps = []
    for b, hh in _shards():
        shard = (
            video[b, :, :, hh * HH : (hh + 1) * HH, :]
            .reshape(T, C, N)
            .astype(np.float16)
        )
        in_maps.append({"x": shard, "wzr": wzr, "wc": wc, "bzr": bzr, "bc": bc})

    from concourse.bass_utils import run_bass_kernel_spmd

    res = run_bass_kernel_spmd(nc, in_maps, core_ids=list(range(NCORES)))
    _CACHE["last_results"] = res

    out = np.zeros((B, T, C, H, W), np.float32)
    for ci, (b, hh) in enumerate(_shards()):
        y = res.results[ci]["y"].astype(np.float32).reshape(T, C, HH, W)
        out[b, :, :, hh * HH : (hh + 1) * HH, :] = y
    return out


# revision 20
# speedup vs baseline: 1.0030x; 1.0030x over previous
"""ConvGRU Trainium2 kernel (8 NeuronCores, data-parallel over batch x H-half).

v3: pair-granular schedule (2 pipeline stages of 784 px), multi-bank PSUM
tiles with 3D-AP activations, col-paired M=64 c-gate matmuls, partition-offset
engine ops instead of realign DMAs, batched DMA, and explicitly staged
per-engine emission order (engines are in-order; program order must match
operand readiness to avoid head-of-line stalls).

Layout (per core shard: batch b, H rows [hh*28, hh*28+28), N = 28*56 = 1568 px):
  Channels on SBUF partitions, pixels on the free dim. C=192 = 128 "low"
  (p0:128 of full tiles) + 64 "high" (p64:128 of mixed tiles).
  Mixed tiles: M1 = [x128:192 @p0:64 ; h128:192 @p64:128],
               M2 = [x128:192 @p0:64 ; rh128:192 @p64:128].
  zr matmul M-tiles: m0=z0:128, m1=r0:128, m2=[z128:192 @p0:64; r128:192 @p64:128].
  c-gate: psa (M=128, c0:128); cb of the two lanes of a pair col-paired into
  ONE bank: lane A -> p64:128 (tile_position (0,64)), lane B -> p0:64 ((0,0)).
  z128:192 (at p0:64 of ZRB) is re-aligned to p64:128 once per pair by a Pool
  tensor_copy with output partition offset.

PSUM budget (8 banks): shared zr+ca pool 3x[128,2,512] (6 banks) + cb pool
2x[128,512] (2 banks).

All on-device tensors fp16 (PSUM fp32); rel err vs fp32 reference ~1.6e-3.
"""

import sys

sys.path.insert(0, "/opt/trn_rl_repo")

import numpy as np

B, T, C, H, W = 4, 16, 192, 56, 56
HH = 28          # H rows per shard
N = HH * W       # 1568 pixels per shard per step
LANE = 392       # pixels per matmul N-chunk (one PSUM bank holds 512 fp32)
PAIR = 2 * LANE  # 784: elementwise/ACT granularity
NCORES = 8

_CACHE = {}


def _build_nc(n_steps):
    from concourse import bacc
    import concourse.mybir as mybir
    import concourse.tile as tile

    F16, F32 = mybir.dt.float16, mybir.dt.float32
    AF = mybir.ActivationFunctionType

    nc = bacc.Bacc(None, target_bir_lowering=False)
    x_d = nc.dram_tensor("x", [n_steps, C, N], F16, kind="ExternalInput")
    wzr_d = nc.dram_tensor("wzr", [128, 3, 3, 128], F16, kind="ExternalInput")
    wc_d = nc.dram_tensor("wc", [128, 3, 192], F16, kind="ExternalInput")
    bzr_d = nc.dram_tensor("bzr", [128, 3], F32, kind="ExternalInput")
    bc_d = nc.dram_tensor("bc", [128, 2], F32, kind="ExternalInput")
    y_d = nc.dram_tensor("y", [n_steps, C, N], F16, kind="ExternalOutput")

    with tile.TileContext(nc) as tc:
        with (
            tc.tile_pool(name="const", bufs=1) as constp,
            tc.tile_pool(name="state", bufs=1) as statep,
            tc.tile_pool(name="pszr", bufs=3, space="PSUM") as pszr,
            tc.tile_pool(name="pscb", bufs=2, space="PSUM") as pscb,
        ):
            # load order: what the first matmuls/ACTs need first.
            # wzr split per M-tile (m1 first): the first Ldweights only
            # needs the m1 slice.
            wzr = constp.tile([128, 3, 3, 128], F16)
            for m in (1, 2, 0):
                nc.sync.dma_start(wzr[:, :, m, :], wzr_d[:, :, m, :])

            Xb = [
                statep.tile([128, N], F16, name="X0a"),
                statep.tile([128, N], F16, name="X0b"),
            ]
            H0 = statep.tile([128, N], F16, name="H0")    # h 0:128
            M1 = statep.tile([128, N], F16, name="M1")    # [x-up; h-up]
            M2 = statep.tile([128, N], F16, name="M2")    # [x-up; rh-up]
            Z0 = statep.tile([128, N], F16, name="Z0")    # z 0:128
            R0 = statep.tile([128, N], F16, name="R0")    # r 0:128
            # ZRB per (parity, pair): the Pool z-copy reads it late, and the
            # dep tracker is conservative across column ranges — separate
            # tiles kill both the WAR edge and the false cross-pair dep
            ZRBb = [
                [
                    statep.tile([128, PAIR], F16, name=f"ZRB{i}{p}")
                    for p in range(2)
                ]
                for i in range(2)
            ]
            ZU = statep.tile([128, N], F16, name="ZU")    # z-up aligned @p64:128
            RH0 = statep.tile([128, N], F16, name="RH0")  # r*h 0:128
            CA = statep.tile([128, N], F16, name="CA")    # c 0:128
            UC = statep.tile([128, N], F16, name="UC")    # c-up @p64:128
            D = statep.tile([128, N], F16, name="D")      # scratch (low)
            EU = statep.tile([128, N], F16, name="EU")    # scratch (up)

            nc.gpsimd.dma_start(Xb[0][:], x_d[0, 0:128, :])
            nc.gpsimd.dma_start(M1[0:64, :], x_d[0, 128:192, :])

            bzr = constp.tile([128, 3], F32)
            nc.sync.dma_start(bzr[:], bzr_d[:])
            wc = constp.tile([128, 3, 192], F16)
            nc.sync.dma_start(wc[:], wc_d[:])
            nc.sync.dma_start(M2[0:64, :], x_d[0, 128:192, :])
            bc = constp.tile([128, 2], F32)
            nc.sync.dma_start(bc[:], bc_d[:])

            nc.vector.memset(H0[:], 0.0)
            nc.vector.memset(M1[64:128, :], 0.0)
            nc.vector.memset(M2[64:128, :], 0.0)

            def zr_phase(t, p, X0, ZRB):
                p0 = p * PAIR
                psl = slice(p0, p0 + PAIR)
                lsl = [slice(p0, p0 + LANE), slice(p0 + LANE, p0 + PAIR)]
                # M-tile order (m1, m2, m0): r-activations drain first — they
                # feed rh which gates the c-phase; z (m0) is only needed at
                # the late h-update.
                for m in (1, 2, 0):
                    ps = pszr.tile([128, 2, 512], F32, tag="zrca")
                    # t=0: h is zero and ready (memset) while x still loads —
                    # lead with H0 zero-chunks so PE warms up during the DMA
                    korder = ((0, H0), (1, M1), (2, X0)) if t == 0 else (
                        (2, X0), (0, H0), (1, M1))
                    first, last = korder[0][1], korder[-1][1]
                    for k, src in korder:
                        for li in range(2):
                            nc.tensor.matmul(
                                ps[:, li, 0:LANE],
                                wzr[:, k, m, :],
                                src[:, lsl[li]],
                                start=(src is first),
                                stop=(src is last),
                                skip_group_check=True,
                            )
                    if m == 2:
                        nc.scalar.activation(
                            ZRB[:, :], ps[:, :, 0:LANE], AF.Sigmoid,
                            bias=bzr[:, 2:3], scale=1.0,
                        )
                    else:
                        dst = (Z0, R0)[m]
                        nc.scalar.activation(
                            dst[:, psl], ps[:, :, 0:LANE], AF.Sigmoid,
                            bias=bzr[:, m : m + 1], scale=1.0,
                        )
                if p == 1 and t + 1 < n_steps:
                    # all zr readers of M1[0:64] emitted; reload x-half for t+1
                    nc.sync.dma_start(M1[0:64, :], x_d[t + 1, 128:192, :])

            def rh_phase(t, p, ZRB):
                p0 = p * PAIR
                psl = slice(p0, p0 + PAIR)
                if t > 0:
                    # at t=0 h==0 so rh==0 (M2 upper half is pre-zeroed)
                    nc.vector.tensor_mul(
                        out=RH0[:, psl], in0=R0[:, psl], in1=H0[:, psl]
                    )
                    nc.vector.tensor_mul(
                        out=M2[64:128, psl], in0=ZRB[64:128, :],
                        in1=M1[64:128, psl],
                    )
                if t + 1 < n_steps:
                    # z-up realign p0:64 -> p64:128. DVE (468ns) beats Pool
                    # (1.18us + sem hops) on the h-update chain; DVE has
                    # headroom. Last step uses the PSUM-d variant instead.
                    nc.vector.tensor_copy(out=ZU[64:128, psl], in_=ZRB[0:64, :])

            def ca_act(p, psa):
                p0 = p * PAIR
                psl = slice(p0, p0 + PAIR)
                nc.scalar.activation(
                    CA[:, psl], psa[:, :, 0:LANE], AF.Tanh,
                    bias=bc[:, 0:1], scale=1.0,
                )

            def cb_acts(p, psb):
                p0 = p * PAIR
                lsl = [slice(p0, p0 + LANE), slice(p0 + LANE, p0 + PAIR)]
                nc.scalar.activation(
                    UC[64:128, lsl[0]], psb[64:128, 0:LANE], AF.Tanh,
                    bias=bc[64:128, 1:2], scale=1.0,
                )
                nc.scalar.activation(
                    UC[64:128, lsl[1]], psb[0:64, 0:LANE], AF.Tanh,
                    bias=bc[0:64, 1:2], scale=1.0,
                )

            def c_mms(t, p, X0):
                p0 = p * PAIR
                lsl = [slice(p0, p0 + LANE), slice(p0 + LANE, p0 + PAIR)]
                psa = pszr.tile([128, 2, 512], F32, tag="zrca")
                psb = pscb.tile([128, 512], F32, tag="cb")
                korder = ((2, X0), (1, M2)) if t == 0 else (
                    (2, X0), (0, RH0), (1, M2))
                for k, src in korder:
                    for li in range(2):
                        rhs = src[:, lsl[li]]
                        nc.tensor.matmul(
                            psa[:, li, 0:LANE], wc[:, k, 0:128], rhs,
                            start=(src is X0), stop=(src is M2),
                            skip_group_check=True,
                        )
                        if li == 0:
                            nc.tensor.matmul(
                                psb[64:128, 0:LANE], wc[:, k, 128:192], rhs,
                                start=(src is X0), stop=(src is M2),
                                skip_group_check=True,
                                tile_position=(0, 64),
                            )
                        else:
                            nc.tensor.matmul(
                                psb[0:64, 0:LANE], wc[:, k, 128:192], rhs,
                                start=(src is X0), stop=(src is M2),
                                skip_group_check=True,
                                tile_position=(0, 0),
                            )
                if p == 1 and t + 1 < n_steps:
                    # c readers of M2[0:64] emitted; reload x-half for t+1
                    nc.sync.dma_start(M2[0:64, :], x_d[t + 1, 128:192, :])
                return psa, psb

            def h_low(p):
                p0 = p * PAIR
                psl = slice(p0, p0 + PAIR)
                nc.vector.tensor_sub(
                    out=D[:, psl], in0=CA[:, psl], in1=H0[:, psl]
                )
                nc.vector.tensor_mul(
                    out=D[:, psl], in0=Z0[:, psl], in1=D[:, psl]
                )
                nc.vector.tensor_add(
                    out=H0[:, psl], in0=H0[:, psl], in1=D[:, psl]
                )

            def h_up_sub(p):
                p0 = p * PAIR
                psl = slice(p0, p0 + PAIR)
                nc.vector.tensor_sub(
                    out=EU[64:128, psl], in0=UC[64:128, psl],
                    in1=M1[64:128, psl],
                )

            def h_up_fin(p):
                p0 = p * PAIR
                psl = slice(p0, p0 + PAIR)
                nc.vector.tensor_mul(
                    out=EU[64:128, psl], in0=ZU[64:128, psl],
                    in1=EU[64:128, psl],
                )
                nc.vector.tensor_add(
                    out=M1[64:128, psl], in0=M1[64:128, psl],
                    in1=EU[64:128, psl],
                )

            def h_up_psumd(p, psb, ZRB):
                # tail variant: d lives in psb (PSUM) so the z-up multiply is
                # a legal mixed-space op — no Pool z-copy in the chain
                p0 = p * PAIR
                psl = slice(p0, p0 + PAIR)
                lsl = [slice(p0, p0 + LANE), slice(p0 + LANE, p0 + PAIR)]
                for li, pr in ((0, slice(64, 128)), (1, slice(0, 64))):
                    nc.vector.tensor_sub(
                        out=psb[pr, 0:LANE], in0=UC[64:128, lsl[li]],
                        in1=M1[64:128, lsl[li]],
                    )
                    nc.vector.tensor_mul(
                        out=EU[64:128, lsl[li]], in0=psb[pr, 0:LANE],
                        in1=ZRB[0:64, li * LANE:(li + 1) * LANE],
                    )
                nc.vector.tensor_add(
                    out=M1[64:128, psl], in0=M1[64:128, psl],
                    in1=EU[64:128, psl],
                )

            for t in range(n_steps):
                X0 = Xb[t % 2]
                if t + 1 < n_steps:
                    nc.sync.dma_start(Xb[(t + 1) % 2][:], x_d[t + 1, 0:128, :])

                zr_phase(t, 0, X0, ZRBb[t % 2][0])
                rh_phase(t, 0, ZRBb[t % 2][0])
                psa0, psb0 = c_mms(t, 0, X0)
                ca_act(0, psa0)
                zr_phase(t, 1, X0, ZRBb[t % 2][1])
                rh_phase(t, 1, ZRBb[t % 2][1])
                cb_acts(0, psb0)
                h_low(0)
                if t + 1 < n_steps:
                    h_up_sub(0)
                    h_up_fin(0)
                else:
                    h_up_psumd(0, psb0, ZRBb[t % 2][0])
                psa1, psb1 = c_mms(t, 1, X0)
                ca_act(1, psa1)
                cb_acts(1, psb1)
                h_low(1)
                if t + 1 < n_steps:
                    h_up_sub(1)
                    h_up_fin(1)
                else:
                    h_up_psumd(1, psb1, ZRBb[t % 2][1])

                if t + 1 < n_steps:
                    nc.sync.dma_start(y_d[t, 0:128, :], H0[:])
                    nc.sync.dma_start(y_d[t, 128:192, :], M1[64:128, :])
                else:
                    # final step: stream out per pair to shorten the tail
                    for p in range(2):
                        psl = slice(p * PAIR, (p + 1) * PAIR)
                        nc.sync.dma_start(y_d[t, 0:128, psl], H0[:, psl])
                        nc.sync.dma_start(y_d[t, 128:192, psl], M1[64:128, psl])

    nc.finalize()
    return nc


def _prep_weights(w_z, w_r, w_h, b_z, b_r, b_h):
    """Host-side weight/bias packing to match the device layout."""
    wz = np.asarray(w_z, np.float32)
    wr = np.asarray(w_r, np.float32)
    wh = np.asarray(w_h, np.float32)

    def k_blocks(Wm):
        # K-tile order (H-part, mixed, X-part) matching rhs tiles (H0, M1, X0)
        return [
            Wm[:, 192:320],
            np.concatenate([Wm[:, 128:192], Wm[:, 320:384]], axis=1),
            Wm[:, 0:128],
        ]

    m_blocks = [
        wz[0:128],
        wr[0:128],
        np.concatenate([wz[128:192], wr[128:192]], axis=0),
    ]
    wzr = np.zeros((128, 3, 3, 128), np.float16)
    for m, Wm in enumerate(m_blocks):
        for k, Wk in enumerate(k_blocks(Wm)):
            wzr[:, k, m, :] = Wk.T.astype(np.float16)

    wc = np.zeros((128, 3, 192), np.float16)
    for k, Wk in enumerate(k_blocks(wh)):
        wc[:, k, :] = Wk.T.astype(np.float16)

    bzr = np.zeros((128, 3), np.float32)
    bzr[:, 0] = b_z[0:128]
    bzr[:, 1] = b_r[0:128]
    bzr[0:64, 2] = b_z[128:192]
    bzr[64:128, 2] = b_r[128:192]
    bc = np.zeros((128, 2), np.float32)
    bc[:, 0] = b_h[0:128]
    bc[0:64, 1] = b_h[128:192]
    bc[64:128, 1] = b_h[128:192]
    return wzr, wc, bzr, bc


def _shards():
    return [(b, hh) for b in range(B) for hh in range(2)]


def kernel(**inputs):
    video = np.asarray(inputs["video"], np.float32)
    wzr, wc, bzr, bc = _prep_weights(
        inputs["w_z"], inputs["w_r"], inputs["w_h"],
        np.asarray(inputs["b_z"], np.float32),
        np.asarray(inputs["b_r"], np.float32),
        np.asarray(inputs["b_h"], np.float32),
    )

    if "nc" not in _CACHE:
        _CACHE["nc"] = _build_nc(T)
    nc = _CACHE["nc"]

    in_maps = []
    for b, hh in _shards():
        shard = (
            video[b, :, :, hh * HH : (hh + 1) * HH, :]
            .reshape(T, C, N)
            .astype(np.float16)
        )
        in_maps.append({"x": shard, "wzr": wzr, "wc": wc, "bzr": bzr, "bc": bc})

    from concourse.bass_utils import run_bass_kernel_spmd

    res = run_bass_kernel_spmd(nc, in_maps, core_ids=list(range(NCORES)))
    _CACHE["last_results"] = res

    out = np.zeros((B, T, C, H, W), np.float32)
    for ci, (b, hh) in enumerate(_shards()):
        y = res.results[ci]["y"].astype(np.float32).reshape(T, C, HH, W)
        out[b, :, :, hh * HH : (hh + 1) * HH, :] = y
    return out
